# revision 1
# baseline (speedup 1.0000x reference)
"""Trainium2 Bass kernel for the Griffin-style gated linear recurrence.

Model (matching the jax reference, including its chunked-scan numerics):
    a = sigmoid(x @ Wa.T + decay_bias)
    i = sigmoid(x @ Wi.T)
    v = x @ Wv.T
    w = sqrt(max(1 - a*a, 1e-8)) * i * v
    chunked scan (chunk=64): cum_decay = prod of a within chunk;
    weighted = w / max(cum_decay, 1e-10); intra = cum_decay * cumsum(weighted);
    states = intra + cum_decay * carry.

The chunked scan (with its 1e-10 clamp) is algebraically identical to the
single global recurrence
    h[t] = a[t] * h[t-1] + g[t] * w[t],   g[t] = min(1, cd[t] * 1e10)
where cd[t] is the within-chunk running product of a (resetting every 64
steps).  Both cd and h map onto the hardware tensor_tensor_scan op (fp32
state, per-partition recurrence along the free axis).

Sharding: 4 batches x 2 channel-halves = 8 cores, no communication.
Per core: x[b] as [1024, 4096] (transposed on host), weight shard
[1024, 192] (transposed), output [192, 4096] (transposed back on host).
Layout on chip: channels on partitions (groups of 128 + 64), time on the
free axis.  Projections run as float32r matmuls (fp32 operands at
1 cycle/row for N=512) accumulating 8 K-tiles in PSUM.
"""

import sys

if "/opt/trn_rl_repo" not in sys.path:
    sys.path.insert(0, "/opt/trn_rl_repo")

from contextlib import ExitStack

import numpy as np

from concourse import bacc, bass, mybir, tile
from concourse.bass_utils import run_bass_kernel_spmd

B, S = 4, 4096
DM, DR = 1024, 384
DC = DR // 2          # channels per core
CH = 64               # scan chunk size
SB = 512              # sequence block per pipeline step
NB = S // SB
KT = DM // 128        # contraction tiles

F32 = mybir.dt.float32
F32R = mybir.dt.float32r
AFT = mybir.ActivationFunctionType
OP = mybir.AluOpType

# channel groups: (gi, c0, c1)
GROUPS = ((0, 0, 128), (1, 128, DC))

_CACHED_NC = None


def _build_nc():
    nc = bacc.Bacc(trn_type="TRN2")

    xT = nc.dram_tensor("xt", [DM, S], F32R, kind="ExternalInput")
    wT = {
        nm: nc.dram_tensor(f"w{nm}t", [DM, DC], F32R, kind="ExternalInput")
        for nm in ("a", "i", "v")
    }
    bias = nc.dram_tensor("biasa", [DC, 1], F32, kind="ExternalInput")
    out = nc.dram_tensor("out", [DC, S], F32, kind="ExternalOutput")

    with tile.TileContext(nc) as tc, ExitStack() as ctx:
        wp = ctx.enter_context(tc.tile_pool(name="wp", bufs=1))
        cp = ctx.enter_context(tc.tile_pool(name="cp", bufs=1))
        xp = ctx.enter_context(tc.tile_pool(name="xp", bufs=2))
        pp = ctx.enter_context(tc.tile_pool(name="pp", bufs=1, space="PSUM"))
        sp = ctx.enter_context(tc.tile_pool(name="sp", bufs=2))
        hp = ctx.enter_context(tc.tile_pool(name="hp", bufs=2))

        # --- constants -------------------------------------------------
        # f32r end-to-end: DMA moves raw fp32 bytes into f32r tiles; the PE
        # rounds on read.  bacc's move_matmul_waits_to_ldweights handles the
        # multi-wait matmuls this produces.
        w_sb = {}
        for nm in ("a", "i", "v"):
            wt = wp.tile([128, KT, DC], F32R, tag=f"w{nm}")
            nc.sync.dma_start(
                wt[:], wT[nm].rearrange("(k p) c -> p k c", p=128))
            w_sb[nm] = wt

        bias_t = {}
        for gi, c0, c1 in GROUPS:
            bt = cp.tile([c1 - c0, 1], F32, tag=f"bias{gi}")
            nc.sync.dma_start(bt[:], bias[c0:c1, :])
            bias_t[gi] = bt

        # shared read-only zero tile: data1 of the per-chunk cd scans
        zeros = cp.tile([128, CH], F32, tag="zeros")
        nc.vector.memset(zeros[:], 0.0)

        # --- main pipeline over sequence blocks ------------------------
        prev_h = None
        for ib in range(NB):
            s0 = ib * SB

            x_sb = xp.tile([128, KT, SB], F32R, tag="x")
            nc.sync.dma_start(
                x_sb[:],
                xT.rearrange("(k p) s -> p k s", p=128)[:, :, s0:s0 + SB])

            zp = {}
            for nm in ("a", "i", "v"):
                for gi, c0, c1 in GROUPS:
                    z = pp.tile([c1 - c0, SB], F32, tag=f"z{nm}{gi}")
                    for k in range(KT):
                        nc.tensor.matmul(
                            z[:],
                            w_sb[nm][:, k, c0:c1],
                            x_sb[:, k, :],
                            start=(k == 0),
                            stop=(k == KT - 1),
                        )
                    zp[(nm, gi)] = z

            new_h = {}
            for gi, c0, c1 in GROUPS:
                P = c1 - c0
                za, zi, zv = zp[("a", gi)], zp[("i", gi)], zp[("v", gi)]
                bt = bias_t[gi]

                a = sp.tile([P, SB], F32, tag=f"a{gi}")
                it = sp.tile([P, SB], F32, tag=f"i{gi}")
                m = sp.tile([P, SB], F32, tag=f"m{gi}")
                r = sp.tile([P, SB], F32, tag=f"r{gi}")
                u = sp.tile([P, SB], F32, tag=f"u{gi}")
                w = sp.tile([P, SB], F32, tag=f"w{gi}")
                cd = sp.tile([P, SB], F32, tag=f"cd{gi}")
                g = sp.tile([P, SB], F32, tag=f"g{gi}")
                gw = sp.tile([P, SB], F32, tag=f"gw{gi}")
                h = hp.tile([P, SB], F32, tag=f"h{gi}")

                nc.scalar.activation(a[:], za[:], AFT.Sigmoid, bias=bt[:])
                nc.scalar.activation(it[:], zi[:], AFT.Sigmoid)
                nc.vector.tensor_mul(m[:], a[:], a[:])
                # r = sqrt(1 - a*a); 1 - a*a stays well above the reference's
                # 1e-8 floor for every reachable a, so the max() is a no-op.
                nc.scalar.activation(r[:], m[:], AFT.Sqrt, bias=1.0, scale=-1.0)
                nc.vector.tensor_mul(u[:], it[:], zv[:])
                nc.vector.tensor_mul(w[:], r[:], u[:])
                # within-chunk running product of a: one scan per 64-chunk
                for c in range(SB // CH):
                    cs = slice(c * CH, (c + 1) * CH)
                    nc.vector.tensor_tensor_scan(
                        cd[:, cs], a[:, cs], zeros[0:P, :], 1.0,
                        op0=OP.mult, op1=OP.add,
                    )
                # g = min(cd * 1e10, 1) == cd / max(cd, 1e-10)
                nc.vector.tensor_scalar(
                    g[:], cd[:], 1e10, 1.0, op0=OP.mult, op1=OP.min
                )
                nc.vector.tensor_mul(gw[:], g[:], w[:])
                init = 0.0 if prev_h is None else prev_h[gi][:, SB - 1:SB]
                nc.vector.tensor_tensor_scan(
                    h[:], a[:], gw[:], init, op0=OP.mult, op1=OP.add
                )
                nc.sync.dma_start(out[c0:c1, s0:s0 + SB], h[:])
                new_h[gi] = h
            prev_h = new_h

    nc.finalize()
    return nc


def _make_in_maps(x, Wa, Wi, Wv, decay_bias):
    x = np.asarray(x, dtype=np.float32)
    Wa = np.asarray(Wa, dtype=np.float32)
    Wi = np.asarray(Wi, dtype=np.float32)
    Wv = np.asarray(Wv, dtype=np.float32)
    decay_bias = np.asarray(decay_bias, dtype=np.float32)

    in_maps = []
    for b in range(B):
        xTb = np.ascontiguousarray(x[b].T)           # [DM, S]
        for j in range(2):
            c0, c1 = j * DC, (j + 1) * DC
            in_maps.append({
                "xt": xTb,
                "wat": np.ascontiguousarray(Wa[c0:c1].T),
                "wit": np.ascontiguousarray(Wi[c0:c1].T),
                "wvt": np.ascontiguousarray(Wv[c0:c1].T),
                "biasa": np.ascontiguousarray(decay_bias[c0:c1, None]),
            })
    return in_maps


def kernel(x, Wa, Wi, Wv, decay_bias):
    global _CACHED_NC
    if _CACHED_NC is None:
        _CACHED_NC = _build_nc()
    nc = _CACHED_NC

    in_maps = _make_in_maps(x, Wa, Wi, Wv, decay_bias)
    res = run_bass_kernel_spmd(nc, in_maps, core_ids=list(range(8)))

    out = np.empty((B, S, DR), dtype=np.float32)
    for b in range(B):
        for j in range(2):
            core = 2 * b + j
            out[b, :, j * DC:(j + 1) * DC] = res.results[core]["out"].T
    return out

